# revision 1
# baseline (speedup 1.0000x reference)
"""AAGA (agent attention) Trainium2 kernel, data-parallel over batch B=8 on 8 NeuronCores.

Per batch b (d=256, K=64 agents, N=8192 tokens), with all weight-only products
folded on the host:
  s1 = x @ (q_agent Wk).T * sc (+bias)   -> softmax over N -> attn1
  v1f2 = (((attn1 @ [x|1]) -> /Z1) Wv.T+bv) Wfc1.T+b1) Wfc2.T+b2
  s2 = (k_agent Wq) @ x.T * sc (+bias)   -> softmax over K -> attn2
  out = rmsnorm(attn2 @ v1f2 + x, norm_scale)

Device design notes (driven by TimelineSim engine/dispatch profiling):
 - x arrives in three host-prepared forms: f32 token-major (exact residual),
   bf16 token-major with a ones column (stage-1 values + Z1 via ones-column),
   and bf16 host-TRANSPOSED (feature-major) -> zero on-device transposes/casts.
 - stage-1 scores are computed token-major per 128-token tile (s1T), exp'd
   directly into the PV lhsT orientation; Z1 rides the ones column of x_aug.
 - stage-2 scores are computed agent-major per 512-token slab (2 matmuls/slab),
   exp'd into a resident [64, 8192] buffer that phase C uses directly as the
   v2 lhsT; Z2 rides a ones column appended to v1f2.
 - softmax normalizers and rmsnorm use batched [128, 64] stats ops; the only
   ACT functions are Exp/Identity (one table) + a single Sqrt at the end.
"""

import os
import sys

import numpy as np

for _p in ("/opt/trn_rl_repo", "/opt/pypackages"):
    if os.path.isdir(_p) and _p not in sys.path:
        sys.path.append(_p)

import ml_dtypes

import concourse.bass as bass
import concourse.bacc as bacc
import concourse.mybir as mybir
import concourse.tile as tile

F32 = mybir.dt.float32
B16 = mybir.dt.bfloat16
AF = mybir.ActivationFunctionType
ALU = mybir.AluOpType
AX = mybir.AxisListType

B, N, D, K = 8, 8192, 256, 64
NT = N // 128       # 64 token tiles
NSLAB = N // 512    # 16 slabs
SC = float(D) ** -0.5
EPS = 1e-8

_CACHE = {}


def _build(apply_s1b: bool, apply_ns: bool) -> bass.Bass:
    nc = bacc.Bacc("TRN2", target_bir_lowering=False, debug=False, num_devices=8)

    x_ext = nc.declare_dram_parameter("x", [N, D], F32, isOutput=False)
    xaug_ext = nc.declare_dram_parameter("xaug_b16", [N, D + 1], B16, isOutput=False)
    xt_ext = nc.declare_dram_parameter("xt_b16", [D, N], B16, isOutput=False)
    qk_d = nc.declare_dram_parameter("qk_eff_t", [128, 2, K], B16, isOutput=False)
    wq_d = nc.declare_dram_parameter("wq_eff_t", [128, 2, K], B16, isOutput=False)
    s2b_d = nc.declare_dram_parameter("s2_bias", [K, 1], F32, isOutput=False)
    s1b_d = nc.declare_dram_parameter("s1_bias_row", [1, K], B16, isOutput=False)
    onr_d = nc.declare_dram_parameter("ones_row", [1, 128], B16, isOutput=False)
    wv_d = nc.declare_dram_parameter("wv_t", [128, 2, D], B16, isOutput=False)
    wf1_d = nc.declare_dram_parameter("wfc1_t", [128, 2, D], B16, isOutput=False)
    wf2_d = nc.declare_dram_parameter("wfc2_t", [128, 2, D], B16, isOutput=False)
    br_d = nc.declare_dram_parameter("b_rows", [1, 3 * D], B16, isOutput=False)
    idf_d = nc.declare_dram_parameter("ident_f32", [128, 128], F32, isOutput=False)
    on_d = nc.declare_dram_parameter("ones_col", [1, K], B16, isOutput=False)
    ns_d = nc.declare_dram_parameter("ns_bcast4", [128, 4, D], F32, isOutput=False)
    out_ext = nc.declare_dram_parameter("out", [N, D], F32, isOutput=True)

    with tile.TileContext(nc, num_cores=8) as tc:
        with tc.tile_pool(name="const", bufs=1) as cpool:
            def load(name, shape, dt, src):
                t = cpool.tile(shape, dt, name=name)
                nc.sync.dma_start(out=t[:], in_=src[:])
                return t

            qk_sb = load("qk_sb", [128, 2, K], B16, qk_d)
            wq_sb = load("wq_sb", [128, 2, K], B16, wq_d)
            s2b_sb = load("s2b_sb", [K, 1], F32, s2b_d)
            s1b_sb = load("s1b_sb", [1, K], B16, s1b_d)
            onr_sb = load("onr_sb", [1, 128], B16, onr_d)
            wv_sb = load("wv_sb", [128, 2, D], B16, wv_d)
            wf1_sb = load("wf1_sb", [128, 2, D], B16, wf1_d)
            wf2_sb = load("wf2_sb", [128, 2, D], B16, wf2_d)
            br_sb = load("br_sb", [1, 3 * D], B16, br_d)
            idf_sb = load("idf_sb", [128, 128], F32, idf_d)
            on_sb = load("on_sb", [1, K], B16, on_d)
            ns_sb = load("ns_sb", [128, 4, D], F32, ns_d)

            # residents
            xsb = cpool.tile([128, NT, D], F32, name="xsb")       # exact x; becomes y
            xaug = cpool.tile([128, NT, D + 1], B16, name="xaug")  # bf16 x | ones
            xT0 = cpool.tile([128, N], B16, name="xT0")
            xT1 = cpool.tile([128, N], B16, name="xT1")
            e2T = cpool.tile([K, N], B16, name="e2T")             # exp(s2T), resident
            v1f2 = cpool.tile([K, D + 1], B16, name="v1f2")       # per-agent vals | 1
            stats = cpool.tile([128, NT], F32, name="stats")      # sum(y^2) per tile
            r3all = cpool.tile([128, NT], F32, name="r3all")      # 1/rms per tile

            # x loads: token-major slabs; xT halves in chunks for pipelining
            for s in range(NSLAB):
                nc.sync.dma_start(
                    out=xsb[:, 4 * s:4 * (s + 1), :],
                    in_=x_ext[512 * s:512 * (s + 1), :].rearrange(
                        "(t p) d -> p t d", p=128
                    ),
                )
                nc.sync.dma_start(
                    out=xaug[:, 4 * s:4 * (s + 1), :],
                    in_=xaug_ext[512 * s:512 * (s + 1), :].rearrange(
                        "(t p) d -> p t d", p=128
                    ),
                )
            NCH = 4
            CW = N // NCH
            for ch in range(NCH):
                nc.sync.dma_start(
                    out=xT0[:, ch * CW:(ch + 1) * CW],
                    in_=xt_ext[0:128, ch * CW:(ch + 1) * CW],
                )
                nc.sync.dma_start(
                    out=xT1[:, ch * CW:(ch + 1) * CW],
                    in_=xt_ext[128:256, ch * CW:(ch + 1) * CW],
                )

            # ---------------- Phase A: stage-1 + stage-2 scores/exp ----------------
            with tc.tile_pool(name="pA_s1", bufs=3, space="PSUM") as ps_s1, \
                 tc.tile_pool(name="pA_s2", bufs=2, space="PSUM") as ps_s2, \
                 tc.tile_pool(name="pA_u1", bufs=1, space="PSUM") as ps_u1, \
                 tc.tile_pool(name="pA_sb", bufs=3) as pa:
                u1 = ps_u1.tile([K, D + 1], F32, name="u1")
                for s in range(NSLAB):
                    # stage-2: s2T [64, 512] for this slab, exp -> e2T resident
                    s2p = ps_s2.tile([K, 512], F32, name="s2p", tag="s2p")
                    for c in range(2):
                        xTc = xT0 if c == 0 else xT1
                        nc.tensor.matmul(
                            s2p[:],
                            wq_sb[:, c, :],
                            xTc[:, 512 * s:512 * (s + 1)],
                            start=(c == 0),
                            stop=(c == 1),
                        )
                    nc.scalar.activation(
                        e2T[:, 512 * s:512 * (s + 1)], s2p[:], AF.Exp,
                        bias=s2b_sb[:], scale=SC,
                    )
                    # stage-1: per 128-token tile
                    for j in range(4):
                        t = 4 * s + j
                        s1p = ps_s1.tile([128, K], F32, name="s1p", tag="s1p")
                        for c in range(2):
                            xTc = xT0 if c == 0 else xT1
                            nc.tensor.matmul(
                                s1p[:],
                                xTc[:, 128 * t:128 * (t + 1)],
                                qk_sb[:, c, :],
                                start=(c == 0),
                                stop=(not apply_s1b and c == 1),
                            )
                        if apply_s1b:
                            nc.tensor.matmul(
                                s1p[:], onr_sb[:], s1b_sb[:],
                                start=False, stop=True,
                            )
                        e1t = pa.tile([128, K], B16, name="e1t", tag="e1t")
                        nc.scalar.activation(e1t[:], s1p[:], AF.Exp, scale=SC)
                        nc.tensor.matmul(
                            u1[:],
                            e1t[:],
                            xaug[:, t, :],
                            start=(t == 0),
                            stop=(t == NT - 1),
                        )

            # ---------------- Phase B: v1f2 chain ----------------
            with tc.tile_pool(name="pB_ps", bufs=2, space="PSUM") as psb, \
                 tc.tile_pool(name="pB_sb", bufs=2) as pb:
                r1 = pb.tile([K, 1], F32, name="r1")
                nc.vector.reciprocal(r1[:], u1[:, D:D + 1])
                cur = pb.tile([K, D], F32, name="chain0")
                nc.scalar.activation(cur[:], u1[:, 0:D], AF.Identity, scale=r1[:])
                for wi, (w_sb, boff) in enumerate(
                    [(wv_sb, 0), (wf1_sb, D), (wf2_sb, 2 * D)]
                ):
                    vT = pb.tile([128, 2, K], B16, name=f"vT{wi}", tag="vT")
                    for c in range(2):
                        tp = psb.tile([128, K], F32, name=f"tp{wi}{c}", tag="tp")
                        nc.tensor.transpose(
                            tp[:], cur[:, 128 * c:128 * (c + 1)], idf_sb[:K, :K]
                        )
                        nc.scalar.activation(vT[:, c, :], tp[:], AF.Identity)
                    nxt = psb.tile([K, D], F32, name=f"ch{wi}", tag="chps")
                    for c in range(2):
                        nc.tensor.matmul(
                            nxt[:], vT[:, c, :], w_sb[:, c, :],
                            start=(c == 0), stop=False,
                        )
                    nc.tensor.matmul(
                        nxt[:], on_sb[:], br_sb[:, boff:boff + D],
                        start=False, stop=True,
                    )
                    if wi == 2:
                        nc.scalar.activation(v1f2[:, 0:D], nxt[:], AF.Identity)
                    else:
                        dst = pb.tile([K, D], F32, name=f"chain{wi + 1}",
                                      tag=f"chain{wi + 1}")
                        nc.scalar.activation(dst[:], nxt[:], AF.Identity)
                        cur = dst
                nc.gpsimd.memset(v1f2[:, D:D + 1], 1.0)

            # ---------------- Phase C1: v2, y, stats ----------------
            with tc.tile_pool(name="pC_v2", bufs=2, space="PSUM") as ps_v2, \
                 tc.tile_pool(name="pC_sb", bufs=3) as pc:
                for s in range(NSLAB):
                    v2p = ps_v2.tile([128, 4, 512], F32, name="v2p", tag="v2p")
                    for j in range(4):
                        t = 4 * s + j
                        nc.tensor.matmul(
                            v2p[:, j, 0:D + 1],
                            e2T[:, 128 * t:128 * (t + 1)],
                            v1f2[:],
                            start=True,
                            stop=True,
                        )
                    r2s = pc.tile([128, 4], F32, name="r2s", tag="r2s")
                    nc.vector.reciprocal(r2s[:], v2p[:, :, D])
                    ysc = pc.tile([128, 4, D], F32, name="ysc", tag="ysc")
                    for j in range(4):
                        nc.scalar.activation(
                            ysc[:, j, :], v2p[:, j, 0:D], AF.Identity,
                            scale=r2s[:, j:j + 1],
                        )
                    # y (in place over x), then q2 and per-tile sum(y^2)
                    nc.vector.tensor_tensor(
                        out=xsb[:, 4 * s:4 * (s + 1), :],
                        in0=ysc[:],
                        in1=xsb[:, 4 * s:4 * (s + 1), :],
                        op=ALU.add,
                    )
                    q2 = pc.tile([128, 4, D], F32, name="q2", tag="q2")
                    nc.vector.tensor_tensor(
                        out=q2[:],
                        in0=xsb[:, 4 * s:4 * (s + 1), :],
                        in1=xsb[:, 4 * s:4 * (s + 1), :],
                        op=ALU.mult,
                    )
                    nc.vector.tensor_reduce(
                        stats[:, 4 * s:4 * (s + 1)], q2[:], axis=AX.X, op=ALU.add
                    )
                    # rms + output for this slab (Sqrt/Identity share one ACT
                    # table; eps=1e-8 is far below f32 noise -> folded away)
                    rms = pc.tile([128, 4], F32, name="rms", tag="rms")
                    nc.scalar.activation(
                        rms[:], stats[:, 4 * s:4 * (s + 1)], AF.Sqrt, scale=1.0 / D
                    )
                    nc.vector.reciprocal(r3all[:, 4 * s:4 * (s + 1)], rms[:])
                    ot = pc.tile([128, 4, D], F32, name="ot", tag="ot")
                    for j in range(4):
                        t = 4 * s + j
                        nc.scalar.activation(
                            ot[:, j, :], xsb[:, t, :], AF.Identity,
                            scale=r3all[:, t:t + 1],
                        )
                    if apply_ns:
                        nc.vector.tensor_tensor(
                            out=ot[:], in0=ot[:], in1=ns_sb[:], op=ALU.mult
                        )
                    nc.gpsimd.dma_start(
                        out=out_ext[512 * s:512 * (s + 1), :].rearrange(
                            "(t p) d -> p t d", p=128
                        ),
                        in_=ot[:],
                    )
    nc.compile()
    return nc


def _make_runner(nc):
    """Cached jitted executor (mirrors bass2jax.run_bass_via_pjrt multi-core)."""
    import jax
    import numpy as _np
    from jax.sharding import Mesh, PartitionSpec
    from jax.experimental.shard_map import shard_map
    from concourse import bass2jax as b2j

    b2j.install_neuronx_cc_hook()

    partition_name = nc.partition_id_tensor.name if nc.partition_id_tensor else None
    in_names, out_names, out_avals, zero_shapes = [], [], [], []
    for alloc in nc.m.functions[0].allocations:
        if not isinstance(alloc, mybir.MemoryLocationSet):
            continue
        name = alloc.memorylocations[0].name
        if alloc.kind == "ExternalInput":
            if name != partition_name:
                in_names.append(name)
        elif alloc.kind == "ExternalOutput":
            out_names.append(name)
            shape = tuple(alloc.tensor_shape)
            dtype = mybir.dt.np(alloc.dtype)
            out_avals.append(jax.core.ShapedArray(shape, dtype))
            zero_shapes.append((shape, dtype))
    n_params, n_outs = len(in_names), len(out_avals)
    all_in_names = list(in_names) + list(out_names)
    if partition_name is not None:
        all_in_names.append(partition_name)
    donate = tuple(range(n_params, n_params + n_outs))

    def _body(*args):
        operands = list(args)
        if partition_name is not None:
            operands.append(b2j.partition_id_tensor())
        outs = b2j._bass_exec_p.bind(
            *operands,
            out_avals=tuple(out_avals),
            in_names=tuple(all_in_names),
            out_names=tuple(out_names),
            lowering_input_output_aliases=(),
            sim_require_finite=True,
            sim_require_nnan=True,
            nc=nc,
        )
        return tuple(outs)

    devices = jax.devices()[:B]
    mesh = Mesh(np.asarray(devices), ("core",))
    in_specs = (PartitionSpec("core"),) * (n_params + n_outs)
    out_specs = (PartitionSpec("core"),) * n_outs
    sharded = jax.jit(
        shard_map(_body, mesh=mesh, in_specs=in_specs, out_specs=out_specs,
                  check_rep=False),
        donate_argnums=donate,
        keep_unused=True,
    )

    def run(in_maps):
        per_core = [[_np.asarray(m[name]) for name in in_names] for m in in_maps]
        concat_in = [
            _np.concatenate([per_core[c][i] for c in range(B)], axis=0)
            for i in range(n_params)
        ]
        concat_zeros = [
            _np.zeros((B * sh[0], *sh[1:]), dt) for (sh, dt) in zero_shapes
        ]
        out_arrs = sharded(*concat_in, *concat_zeros)
        return [
            {
                name: _np.asarray(out_arrs[i]).reshape(B, *out_avals[i].shape)[c]
                for i, name in enumerate(out_names)
            }
            for c in range(B)
        ]

    run.sharded = sharded
    run.in_names = in_names
    run.zero_shapes = zero_shapes
    run.out_names = out_names
    run.out_avals = out_avals
    return run


def _get_runner(apply_s1b: bool, apply_ns: bool):
    key = (apply_s1b, apply_ns)
    if key not in _CACHE:
        nc = _build(apply_s1b, apply_ns)
        _CACHE[key] = _make_runner(nc)
    return _CACHE[key]


def _bf16(a):
    return np.ascontiguousarray(np.asarray(a).astype(ml_dtypes.bfloat16))


def prepare(agent, x, W_qkv, b_qkv, W_agent, b_agent, W_fc1, b_fc1, W_fc2, b_fc2,
            norm_scale):
    """Host-side prep: weight folding + layout. Returns (in_maps, flags)."""
    agent = np.asarray(agent, dtype=np.float32)
    x = np.asarray(x, dtype=np.float32)
    W_qkv = np.asarray(W_qkv, dtype=np.float32)
    b_qkv = np.asarray(b_qkv, dtype=np.float32)
    W_agent = np.asarray(W_agent, dtype=np.float32)
    b_agent = np.asarray(b_agent, dtype=np.float32)
    W_fc1 = np.asarray(W_fc1, dtype=np.float32)
    b_fc1 = np.asarray(b_fc1, dtype=np.float32)
    W_fc2 = np.asarray(W_fc2, dtype=np.float32)
    b_fc2 = np.asarray(b_fc2, dtype=np.float32)
    norm_scale = np.asarray(norm_scale, dtype=np.float32)

    qa_ka = agent @ W_agent.T + b_agent
    q_agent, k_agent = qa_ka[:, :D], qa_ka[:, D:]
    Wq, Wk, Wv = W_qkv[:D], W_qkv[D:2 * D], W_qkv[2 * D:]
    b_q, b_k, b_v = b_qkv[:D], b_qkv[D:2 * D], b_qkv[2 * D:]

    qk_eff = q_agent @ Wk                       # [K, D]
    s1_bias = q_agent @ b_k                     # [K] (raw; device scales by sc)
    wq_eff = k_agent @ Wq                       # [K, D]
    s2_bias = (k_agent @ b_q) * SC              # [K] (pre-scaled; ACT bias)
    apply_s1b = bool(np.any(s1_bias != 0.0))
    apply_ns = not bool(np.allclose(norm_scale, 1.0))

    def dmaj(m):  # [K, D] -> [128, 2, K]
        return m.T.reshape(2, 128, -1).transpose(1, 0, 2)

    def rmaj(w):  # [dout, din] -> [128, 2, dout]
        return w.T.reshape(2, 128, -1).transpose(1, 0, 2)

    shared = {
        "qk_eff_t": _bf16(dmaj(qk_eff)),
        "wq_eff_t": _bf16(dmaj(wq_eff)),
        "s2_bias": np.ascontiguousarray(s2_bias.reshape(K, 1), dtype=np.float32),
        "s1_bias_row": _bf16(s1_bias.reshape(1, K)),
        "ones_row": _bf16(np.ones((1, 128))),
        "wv_t": _bf16(rmaj(Wv)),
        "wfc1_t": _bf16(rmaj(W_fc1)),
        "wfc2_t": _bf16(rmaj(W_fc2)),
        "b_rows": _bf16(np.concatenate([b_v, b_fc1, b_fc2]).reshape(1, 3 * D)),
        "ident_f32": np.eye(128, dtype=np.float32),
        "ones_col": _bf16(np.ones((1, K))),
        "ns_bcast4": np.ascontiguousarray(
            np.broadcast_to(norm_scale.reshape(1, 1, D), (128, 4, D)),
            dtype=np.float32,
        ),
    }

    xb = x.astype(ml_dtypes.bfloat16)
    in_maps = []
    for b in range(B):
        xaug = np.empty((N, D + 1), dtype=ml_dtypes.bfloat16)
        xaug[:, :D] = xb[b]
        xaug[:, D] = 1.0
        in_maps.append(dict(
            shared,
            x=np.ascontiguousarray(x[b]),
            xaug_b16=xaug,
            xt_b16=np.ascontiguousarray(xb[b].T),
        ))
    return in_maps, (apply_s1b, apply_ns)


def kernel(agent, x, W_qkv, b_qkv, W_agent, b_agent, W_fc1, b_fc1, W_fc2, b_fc2,
           norm_scale):
    in_maps, flags = prepare(agent, x, W_qkv, b_qkv, W_agent, b_agent,
                             W_fc1, b_fc1, W_fc2, b_fc2, norm_scale)
    runner = _get_runner(*flags)
    results = runner(in_maps)
    out = np.stack([results[b]["out"] for b in range(B)], axis=0)
    return out.astype(np.float32)

